# revision 1
# baseline (speedup 1.0000x reference)
"""Trainium2 Bass kernel for a dense transformer attention block.

Shards across 8 NeuronCores: data-parallel over batch (2) x tensor-parallel
over heads (4 groups of 4 heads).  Each core computes q/k/v projections for
its head group, rotary embedding, causal attention, and its slice of the
output projection; the host sums the 8 partial outputs and adds the bias.
"""

import sys

sys.path.insert(0, "/opt/trn_rl_repo")

import numpy as np
import ml_dtypes

import concourse.bass as bass  # noqa: F401  (ensures package init)
import concourse.mybir as mybir
import concourse.tile as tile
from concourse import bacc
from concourse.bass_utils import run_bass_kernel_spmd

try:
    import jax as _jax
    _jax.config.update("jax_compilation_cache_dir", "/tmp/nn_attn_jax_cache")
    _jax.config.update("jax_persistent_cache_min_compile_time_secs", 0.0)
    _jax.config.update("jax_persistent_cache_min_entry_size_bytes", 0)
except Exception:
    pass

F32 = mybir.dt.float32
F32R = mybir.dt.float32r
BF16 = mybir.dt.bfloat16

NUM_HEADS = 16
DIM_HEAD = 64
ROPE_BASE = 10000.0
B, N, DIM = 2, 2048, 1024
INNER = NUM_HEADS * DIM_HEAD
NCORES = 8
GROUPS = 4                      # head groups (tensor parallel)
H_LOC = NUM_HEADS // GROUPS     # 4 heads per core
FEAT = H_LOC * DIM_HEAD         # 256 features per core
SCALE = DIM ** (-0.5)

NT = N // 128                   # 16 n-tiles of 128
NCH = N // 512                  # 4 n-chunks of 512
KT = DIM // 128                 # 8 contraction tiles


def _build_module(reps=1, loop_reps=None):
    nc = bacc.Bacc("TRN2", target_bir_lowering=False, debug=False,
                   num_devices=NCORES)

    # ---- DRAM I/O ----
    d_xT = nc.dram_tensor("xT", [DIM, N], F32R, kind="ExternalInput")
    d_wq = nc.dram_tensor("wq", [DIM, FEAT], F32R, kind="ExternalInput")
    d_wk = nc.dram_tensor("wk", [DIM, FEAT], F32R, kind="ExternalInput")
    d_ident = nc.dram_tensor("ident", [128, 128], F32R, kind="ExternalInput")
    d_wv = nc.dram_tensor("wv", [DIM, FEAT], F32R, kind="ExternalInput")
    d_wfc = nc.dram_tensor("wfc", [FEAT, DIM], F32R, kind="ExternalInput")
    d_cosN = nc.dram_tensor("cosN", [128, NT * 32], F32, kind="ExternalInput")
    d_sinNp = nc.dram_tensor("sinNp", [128, NT * 32], F32,
                             kind="ExternalInput")
    d_sinNn = nc.dram_tensor("sinNn", [128, NT * 32], F32,
                             kind="ExternalInput")
    d_masks = nc.dram_tensor("masks", [4, 128, 512], F32R, kind="ExternalInput")
    d_y = nc.dram_tensor("y", [N, DIM], F32, kind="ExternalOutput")

    Exp = mybir.ActivationFunctionType.Exp
    from contextlib import nullcontext, ExitStack

    with tile.TileContext(nc) as tc:
        with tc.tile_pool(name="persist", bufs=1) as pers:
            qT = pers.tile([128, 2, N], F32R)      # roped q^T (2 heads/ptile)
            kT = pers.tile([128, 2, N], F32R)
            vaug = pers.tile([128, NT, H_LOC, 65], F32R)  # roped v + ones col
            attn = pers.tile([128, 2, N], F32R)    # normalized attn^T
            wfc = pers.tile([128, 2, DIM], F32R)
            masks = pers.tile([128, 4, 512], F32R)

            ones_sb = pers.tile([128, NT * H_LOC], F32)
            nc.vector.memset(ones_sb, 1.0)
            nc.vector.tensor_copy(
                vaug.rearrange("p nt h c -> p (nt h) c")[:, :, 64:65],
                ones_sb.rearrange("p (a b) -> p a b", b=1))

            loop_cm = (tc.For_i(0, loop_reps, 1, name="bench")
                       if loop_reps else nullcontext())
            with loop_cm:
             for _rep in range(reps):
                att_psum_es = ExitStack()
                # ---------- phase 1: projections ----------
                with tc.tile_pool(name="projc", bufs=1) as pc, \
                     tc.tile_pool(name="ropetmp", bufs=2) as rt, \
                     tc.tile_pool(name="natp", bufs=3) as natp:
                    xT8 = pc.tile([128, KT, N], F32R)
                    d_xT_r = d_xT.rearrange("(kt p) n -> p kt n", p=128)
                    wq8 = pc.tile([128, KT, FEAT], F32R)
                    wk8 = pc.tile([128, KT, FEAT], F32R)
                    wv8 = pc.tile([128, KT, FEAT], F32R)
                    ident = pc.tile([128, 128], F32R)
                    cosN = pc.tile([128, NT, 32], F32)
                    sinNp = pc.tile([128, NT, 32], F32)
                    sinNn = pc.tile([128, NT, 32], F32)
                    nc.sync.dma_start(
                        out=wq8,
                        in_=d_wq.rearrange("(kt p) f -> p kt f", p=128))
                    # x streamed column-major: first n-chunk lands first so
                    # the q pass starts after ~2MB instead of 8MB; rope
                    # tables + identity arrive right after the first chunk
                    for icn in range(NCH):
                        for kt in range(KT):
                            nc.sync.dma_start(
                                out=xT8[:, kt:kt + 1,
                                        icn * 512:(icn + 1) * 512],
                                in_=d_xT_r[:, kt:kt + 1,
                                           icn * 512:(icn + 1) * 512])
                        if icn == 0:
                            nc.sync.dma_start(out=ident, in_=d_ident[:, :])
                            nc.sync.dma_start(
                                out=cosN,
                                in_=d_cosN.rearrange("p (nt f) -> p nt f",
                                                     f=32))
                            nc.sync.dma_start(
                                out=sinNp,
                                in_=d_sinNp.rearrange("p (nt f) -> p nt f",
                                                      f=32))
                            nc.sync.dma_start(
                                out=sinNn,
                                in_=d_sinNn.rearrange("p (nt f) -> p nt f",
                                                      f=32))

                    def rope_nat(ps_n, nt, out3):
                        """out3[128, 4, 64] (f32r) = rope(ps_n) natural."""
                        v4 = ps_n.rearrange("p (hh two f) -> p hh two f",
                                            two=2, f=32)
                        m1 = rt.tile([128, FEAT], F32, tag="m1", name="m1")
                        m2 = rt.tile([128, FEAT], F32, tag="m2", name="m2")
                        m24 = m2.rearrange("p (hh two f) -> p hh two f",
                                           two=2, f=32)
                        cb = cosN[:, nt, :].rearrange(
                            "p (o f) -> p o f", o=1).to_broadcast([128, 8, 32])
                        nc.vector.tensor_mul(
                            m1.rearrange("p (b f) -> p b f", f=32),
                            ps_n.rearrange("p (b f) -> p b f", f=32), cb)
                        cbn = sinNn[:, nt, :].rearrange(
                            "p (o f) -> p o f", o=1).to_broadcast([128, 4, 32])
                        cbp = sinNp[:, nt, :].rearrange(
                            "p (o f) -> p o f", o=1).to_broadcast([128, 4, 32])
                        nc.vector.tensor_mul(m24[:, :, 0, :],
                                             v4[:, :, 1, :], cbn)
                        nc.vector.tensor_mul(m24[:, :, 1, :],
                                             v4[:, :, 0, :], cbp)
                        nc.gpsimd.tensor_add(
                            out3,
                            m1.rearrange("p (h f) -> p h f", f=64),
                            m2.rearrange("p (h f) -> p h f", f=64))

                    # q/k natural proj + rope, then PE-transpose into qT/kT
                    with tc.tile_pool(name="pn", bufs=3, space="PSUM") as pn, \
                         tc.tile_pool(name="ptr", bufs=3,
                                      space="PSUM") as ptr:
                        for (w8, dstT) in ((wq8, qT), (wk8, kT)):
                            if dstT is kT:
                                nc.sync.dma_start(
                                    out=wk8,
                                    in_=d_wk.rearrange("(kt p) f -> p kt f",
                                                       p=128))
                            for nt in range(NT):
                                ps_n = pn.tile([128, FEAT], F32, tag="ps_n",
                                               name="ps_n")
                                for kt in range(KT):
                                    nc.tensor.matmul(
                                        ps_n,
                                        xT8[:, kt, nt * 128:(nt + 1) * 128],
                                        w8[:, kt, :],
                                        start=(kt == 0), stop=(kt == KT - 1))
                                qnat = natp.tile([128, FEAT], F32R,
                                                 tag="qnat", name="qnat")
                                rope_nat(ps_n, nt,
                                         qnat.rearrange("p (h f) -> p h f",
                                                        f=64))
                                nsl = slice(nt * 128, (nt + 1) * 128)
                                for pp2 in range(2):
                                    ps_t = ptr.tile([128, 128], F32R,
                                                    tag="ps_t", name="ps_t")
                                    nc.tensor.transpose(
                                        ps_t,
                                        qnat[:, pp2 * 128:(pp2 + 1) * 128],
                                        ident)
                                    nc.scalar.copy(dstT[:, pp2, nsl], ps_t)

                    # v projection (natural layout) + rope
                    nc.sync.dma_start(
                        out=wv8,
                        in_=d_wv.rearrange("(kt p) f -> p kt f", p=128))
                    nc.sync.dma_start(
                        out=masks, in_=d_masks.rearrange("m p f -> p m f"))
                    nc.sync.dma_start(
                        out=wfc, in_=d_wfc.rearrange("(t p) d -> p t d", p=128))
                    psc = att_psum_es.enter_context(
                        tc.tile_pool(name="psc", bufs=2, space="PSUM"))
                    pso = att_psum_es.enter_context(
                        tc.tile_pool(name="pso", bufs=1, space="PSUM"))
                    with tc.tile_pool(name="pv", bufs=2, space="PSUM") as pv:
                        for nt in range(NT):
                            ps_v = pv.tile([128, FEAT], F32, tag="ps_v")
                            for kt in range(KT):
                                nc.tensor.matmul(
                                    ps_v, xT8[:, kt, nt * 128:(nt + 1) * 128],
                                    wv8[:, kt, :],
                                    start=(kt == 0), stop=(kt == KT - 1))
                            rope_nat(ps_v, nt, vaug[:, nt, :, 0:64])

                # ---------- phase 2: attention + FC, per i-chunk ----------
                with tc.tile_pool(name="ptp", bufs=3) as ptp, \
                     tc.tile_pool(name="osb", bufs=2) as osb, \
                     tc.tile_pool(name="rrp", bufs=4) as rrp, \
                     tc.tile_pool(name="rbp", bufs=2) as rbp, \
                     tc.tile_pool(name="ysb", bufs=3) as ysb, \
                     tc.tile_pool(name="psf", bufs=2, space="PSUM") as psf:
                    for ic in range(NCH):
                        isl = slice(ic * 512, (ic + 1) * 512)
                        njt = 4 * ic + 4
                        for p in range(2):
                            ps_o = [pso.tile([65, 512], F32, tag=f"ps_o{h}",
                                             name=f"ps_o{h}")
                                    for h in range(2)]
                            # j-tiles in groups of 2 sharing one 2-bank psum
                            # so each Exp covers [128, 1024]
                            for jg in range(njt // 2):
                                dj = 2 * jg - 4 * ic
                                for half in range(2):
                                    hsl = slice(half * 64, (half + 1) * 64)
                                    ps_s = psc.tile([128, 2, 512], F32,
                                                    tag="ps_s", name="ps_s")
                                    for jj in range(2):
                                        jt = 2 * jg + jj
                                        jsl = slice(jt * 128, (jt + 1) * 128)
                                        nc.tensor.matmul(
                                            ps_s[:, jj, :],
                                            kT[hsl, p, jsl], qT[hsl, p, isl],
                                            start=True, stop=True,
                                            tile_position=(half * 64, 0))
                                    pt = ptp.tile([128, 2, 512], F32R,
                                                  tag=f"pt{half}",
                                                  name=f"pt{half}")
                                    nc.scalar.activation(out=pt, in_=ps_s,
                                                         func=Exp,
                                                         scale=SCALE)
                                    if dj >= 0:
                                        nc.gpsimd.tensor_mul(
                                            pt, pt, masks[:, dj:dj + 2, :])
                                    h = 2 * p + half
                                    for jj in range(2):
                                        jt = 2 * jg + jj
                                        nc.tensor.matmul(
                                            ps_o[half], vaug[:, jt, h, :],
                                            pt[:, jj, :],
                                            start=(jt == 0),
                                            stop=(jt == njt - 1))
                            # normalization
                            o_s, rr = [], []
                            for half in range(2):
                                o_sb = osb.tile([65, 512], F32R,
                                                tag=f"o_sb{half}",
                                                name=f"o_sb{half}")
                                nc.vector.tensor_copy(o_sb, ps_o[half])
                                r = rrp.tile([1, 512], F32R, tag=f"rr{half}",
                                             name=f"rr{half}")
                                nc.sync.dma_start(out=r[0:1, :],
                                                  in_=o_sb[64:65, :])
                                with nc.allow_low_precision(
                                        reason="softmax denom recip, f32r"):
                                    nc.vector.reciprocal(r[0:1, :], r[0:1, :])
                                o_s.append(o_sb)
                                rr.append(r)
                            rbc = rbp.tile([128, 512], F32R, tag="rbc",
                                           name="rbc")
                            nc.gpsimd.partition_broadcast(rbc[:, :],
                                                          rr[1][0:1, :])
                            nc.gpsimd.partition_broadcast(rbc[0:64, :],
                                                          rr[0][0:1, :])
                            nc.vector.tensor_mul(attn[0:64, p, isl],
                                                 o_s[0][0:64, :],
                                                 rbc[0:64, :])
                            nc.sync.dma_start(out=attn[64:128, p, isl],
                                              in_=o_s[1][0:64, :])
                            nc.vector.tensor_mul(attn[64:128, p, isl],
                                                 attn[64:128, p, isl],
                                                 rbc[64:128, :])
                        # FC for the 4 i-tiles of this chunk
                        for it in range(4 * ic, 4 * ic + 4):
                            tsl = slice(it * 128, (it + 1) * 128)
                            y_s = ysb.tile([128, 1024], F32, tag="y_s")
                            for dch in range(2):
                                ps_y = psf.tile([128, 512], F32, tag="ps_y",
                                                name="ps_y")
                                for p2 in range(2):
                                    nc.tensor.matmul(
                                        ps_y, attn[:, p2, tsl],
                                        wfc[:, p2,
                                            dch * 512:(dch + 1) * 512],
                                        start=(p2 == 0), stop=(p2 == 1))
                                # DVE only: ACT is saturated by Exp during
                                # the attention phase
                                nc.vector.tensor_copy(
                                    y_s[:, dch * 512:(dch + 1) * 512], ps_y)
                            nc.sync.dma_start(out=d_y[tsl, :], in_=y_s)
                att_psum_es.close()
    nc.compile()
    return nc


_NC = None


def _get_module():
    global _NC
    if _NC is None:
        _NC = _build_module()
    return _NC


def _host_tables():
    inv_freq = 1.0 / (ROPE_BASE ** (np.arange(0, DIM_HEAD, 2,
                                              dtype=np.float32) / DIM_HEAD))
    t = np.arange(N, dtype=np.float32)
    freqs = np.outer(t, inv_freq)            # [N, 32]
    cos = np.cos(freqs).astype(np.float32)
    sin = np.sin(freqs).astype(np.float32)
    cosT = np.ascontiguousarray(np.tile(cos.T, (4, 1)))   # [128, N]
    sinT = np.ascontiguousarray(np.tile(sin.T, (4, 1)))
    # causal 0/1 mask tiles: mask[jj][j, i] = 1 iff i >= jj*128 + j
    i_loc = np.arange(512)
    j_loc = np.arange(128)
    masks = np.zeros((4, 128, 512), np.float32)
    for jj in range(4):
        masks[jj] = (i_loc[None, :] >= (jj * 128 + j_loc)[:, None])
    return cosT, sinT, cos, sin, masks


def _pack_tab(t):
    """[2048, 32] (n, f) -> [128, NT*32] packed row-contiguous."""
    return np.ascontiguousarray(
        t.reshape(NT, 128, 32).transpose(1, 0, 2).reshape(128, NT * 32))


def _make_in_maps(x, Wq, Wk, Wv, Wfc):
    cosT, sinT, cosN, sinN, masks = _host_tables()
    xTs = [np.ascontiguousarray(x[bi].T) for bi in range(B)]
    in_maps = []
    for core in range(NCORES):
        bi, g = core // GROUPS, core % GROUPS
        rs = slice(g * FEAT, (g + 1) * FEAT)
        in_maps.append({
            "xT": xTs[bi],
            "wq": np.ascontiguousarray(Wq[rs].T),
            "wk": np.ascontiguousarray(Wk[rs].T),
            "wv": np.ascontiguousarray(Wv[rs].T),
            "wfc": np.ascontiguousarray(Wfc[:, rs].T),
            "ident": np.eye(128, dtype=np.float32),
            "cosN": _pack_tab(cosN), "sinNp": _pack_tab(sinN),
            "sinNn": _pack_tab(-sinN),
            "masks": masks,
        })
    return in_maps


def _reference_numpy(x, input_mask, Wq, Wk, Wv, Wfc, bfc):
    """Exact fallback for non-trivial input masks."""
    b, n, dim = x.shape
    h, dh = NUM_HEADS, DIM_HEAD
    scale = dim ** (-0.5)
    x64 = x.astype(np.float64)

    def proj(W):
        y = x64 @ W.astype(np.float64).T
        return y.reshape(b, n, h, dh).transpose(0, 2, 1, 3)

    q, k, v = proj(Wq), proj(Wk), proj(Wv)
    inv_freq = 1.0 / (ROPE_BASE ** (np.arange(0, dh, 2) / dh))
    t = np.arange(n)
    freqs = np.outer(t, inv_freq)
    freqs = np.concatenate([freqs, freqs], axis=-1)
    cosf, sinf = np.cos(freqs), np.sin(freqs)

    def rope(u):
        u1, u2 = u[..., :dh // 2], u[..., dh // 2:]
        ru = np.concatenate([-u2, u1], axis=-1)
        return u * cosf + ru * sinf

    q, k, v = rope(q), rope(k), rope(v)
    energy = np.einsum('bhid,bhjd->bhij', q, k) * scale
    mask_value = -np.finfo(np.float32).max
    pm = input_mask[:, None, :, None] & input_mask[:, None, None, :]
    energy = np.where(pm, energy, mask_value)
    causal = np.arange(n)[:, None] < np.arange(n)[None, :]
    energy = np.where(causal[None, None], mask_value, energy)
    energy = energy - energy.max(axis=-1, keepdims=True)
    a = np.exp(energy)
    a = a / a.sum(axis=-1, keepdims=True)
    out = np.einsum('bhij,bhjd->bhid', a, v)
    out = out.transpose(0, 2, 1, 3).reshape(b, n, h * dh)
    return (out @ Wfc.astype(np.float64).T + bfc).astype(np.float32)


def kernel(x, input_mask, Wq, Wk, Wv, Wfc, bfc):
    x = np.asarray(x, dtype=np.float32)
    input_mask = np.asarray(input_mask)
    Wq = np.asarray(Wq, dtype=np.float32)
    Wk = np.asarray(Wk, dtype=np.float32)
    Wv = np.asarray(Wv, dtype=np.float32)
    Wfc = np.asarray(Wfc, dtype=np.float32)
    bfc = np.asarray(bfc, dtype=np.float32)

    if not bool(input_mask.all()):
        return _reference_numpy(x, input_mask, Wq, Wk, Wv, Wfc, bfc)

    nc = _get_module()
    in_maps = _make_in_maps(x, Wq, Wk, Wv, Wfc)

    import os
    trace = os.environ.get("NN_ATTN_TRACE") == "1"
    try:
        res = run_bass_kernel_spmd(nc, in_maps, core_ids=list(range(NCORES)),
                                   trace=trace)
    except ModuleNotFoundError:
        res = run_bass_kernel_spmd(nc, in_maps, core_ids=list(range(NCORES)))
    global last_results
    last_results = res
    y = np.zeros((B, N, DIM), np.float32)
    for core in range(NCORES):
        y[core // GROUPS] += res.results[core]["y"]
    y += bfc
    return y

